# revision 1
# baseline (speedup 1.0000x reference)
"""BinnedColorLoss Trainium2 kernel (v2).

loss = -mean_{b,h,w}[ (sum_k logp[b, idx_k, h, w] * wts_k) * w ]
with logp = log_softmax(pred, axis=1), idx/wts/w gathered per-pixel from
313-entry KNN tables via the pixel's bin t = binned_color[b,0,h,w].

Math restructuring (per pixel, t = bin, lse = logsumexp over C):
  sum_k logp[idx_k]*wts_k*w = (sum_k pred[idx_k]*wts_k)*w - lse * (w*sum_k wts_k)
Let arow(pix) = A[t(pix), :] with A[t,c] = w[t]*sum_k wts[t,k]*[idx[t,k]=c]
(5-sparse rows, gathered per pixel on the HOST into a dense fp8 stream), and
coef(pix) = w[t]*sum_k wts[t,k].  Then with N = B*H*W:
  loss = ( sum_pix lse(pix)*coef(pix) - sum_pix <arow(pix), pred[:,pix]> ) / N

Device strategy (data-parallel over 8 cores, 2 images each; host pre-transposes
to [128 part, 256 chunk, 320 cpad] fp8 with C padded 313->320):
  - ACT: exp of each [128, 16*320] group (fp8 in, f16 out). ~73us, the floor.
  - DVE: per-pixel sumexp via a halving ADD tree in f16 (2x perf mode;
    a plain tensor_reduce would run 1x) -> sume[128, 256]; Ln at the end.
  - PE:  G-term via fp8 DoubleRow matmuls on DIAGONAL C-blocks only:
         S_m += arows[:, 2j:2j+2, off:off+sz].T @ pred[:, 2j:2j+2, off:off+sz]
         (contracts 256 pixels/instr; only diag blocks are needed since the
         contribution is sum_c arows[pix,c]*pred[pix,c]).
  - Tail: diag-mask dots of S_m and the lse.coef dot -> out [128, 8].
Host combines the 8 per-core [128, 8] partials: loss = (L - G)/N.
"""

import sys

for _p in ("/opt/trn_rl_repo",):
    if _p not in sys.path:
        sys.path.insert(0, _p)

from contextlib import ExitStack

import numpy as np

import concourse.bacc as bacc
import concourse.bass as bass  # noqa: F401
import concourse.mybir as mybir
from concourse import bass_utils, tile

F32 = mybir.dt.float32
F16 = mybir.dt.float16
BF16 = mybir.dt.bfloat16
FP8 = mybir.dt.float8e4

B, C, H, W, K = 16, 313, 128, 128, 5
CP = 320                   # C padded (even tree levels, aligned rows)
NCORES = 8
BPC = B // NCORES          # images per core
PIX = BPC * H * W          # pixels per core (32768)
P = 128                    # pixels per chunk (partition dim)
NCHUNK = PIX // P          # 256
G = 16                     # max chunks per group (tile size)
NGRP = NCHUNK // G         # (legacy; schedule uses G_LIST)
NTOT = B * H * W           # mean denominator
CTILES = [(0, 128), (128, 128), (256, 64)]   # diagonal c-blocks of 320
PAD_VAL = -87.0            # exp(pad) == 0
# warm-up/cool-down schedule: small first groups so the first ACT starts
# early, small last groups so the final tree+Ln tail is short
G_LIST = [4, 4, 8] + [16] * 14 + [8, 4, 4]   # chunks per group (sums to 256)
# lse pieces: (first_grp, end_grp, n_chunks, emit_after_grp, out_col).
# Each piece gets its own sume tile so the mid-run Ln has no false deps;
# emit_after leaves a full ACT group of slack after the piece's last reduce.
PIECES = [
    (0, 10, 128, 13, 5),     # chunks [0,128)   ready g9,  Ln after grp 13
    (10, 19, 124, 19, 6),    # chunks [128,252) ready g18, Ln after grp 19
    (19, 20, 4, -1, 0),      # chunks [252,256) final tail (last 4 chunks)
]


def build_program(pe_groups=None):
    """pe_groups: how many of the NGRP groups compute the G-dot on PE
    (fp8 DoubleRow diag matmuls); the rest use a DVE tensor_tensor_reduce."""
    if pe_groups is None:
        import os

        pe_groups = int(os.environ.get("KERNEL_PE_GROUPS", len(G_LIST)))
    nc = bacc.Bacc(
        "TRN2",
        target_bir_lowering=False,
        debug=False,
        enable_asserts=False,
        num_devices=NCORES,
    )
    # Prefer an activation-table set containing BOTH Exp and Ln so the
    # mid-run Ln pieces don't force ~1.3us exp<->ln table re-loads on the
    # (critical) Scalar queue. The table-load pass scans this cached dict
    # in order; moving the combined set first makes it the pick for both.
    import os

    if os.environ.get("KERNEL_TABLE_REORDER", "1") == "1":
        # NOTE: the dict ORDER must stay canonical — act_func_set_id is the
        # index into act_info.json, which NRT resolves by its own copy.
        # Instead, drop Exp/Ln from every other set so the selection pass
        # can only pick the combined set for both functions.
        import concourse.hw_specs as hw_specs

        tabs = hw_specs.get_activation_tables(nc.m.arch)
        _E = mybir.ActivationFunctionType.Exp
        _L = mybir.ActivationFunctionType.Ln
        if any(_E in v and _L in v for v in tabs.values()):
            combined = next(
                k for k, v in tabs.items() if _E in v and _L in v
            )
            for k, v in tabs.items():
                if k != combined:
                    v.discard(_E)
                    v.discard(_L)
    pa_d = nc.dram_tensor(
        "pa_t", [P, NCHUNK, 2, CP], FP8, kind="ExternalInput"
    ).ap()
    coef_d = nc.dram_tensor("coef_t", [P, NCHUNK], F32, kind="ExternalInput").ap()
    mask_d = nc.dram_tensor("mask_t", [P, P], BF16, kind="ExternalInput").ap()
    out_d = nc.dram_tensor("out", [P, 8], F32, kind="ExternalOutput").ap()

    with tile.TileContext(nc) as tc, ExitStack() as ctx, nc.allow_low_precision(
        "f16 exp-sum tree + fp8 G-dot; validated rel err ~1e-3 << 2e-2 tol"
    ):
        const = ctx.enter_context(tc.tile_pool(name="const", bufs=1))
        pap = ctx.enter_context(tc.tile_pool(name="pa", bufs=4))
        expp = ctx.enter_context(tc.tile_pool(name="exp", bufs=3))
        trp = ctx.enter_context(tc.tile_pool(name="tree", bufs=3))
        accp = ctx.enter_context(tc.tile_pool(name="acc", bufs=1))
        psum = ctx.enter_context(tc.tile_pool(name="psum", bufs=1, space="PSUM"))

        ngrp = len(G_LIST)
        starts = [sum(G_LIST[:i]) for i in range(ngrp)]

        # DMA issue management: pred+arows interleaved in one tensor, one
        # transfer per group, issued two groups ahead of the consumer.
        pas = {}

        def issue_pa(g, eng=None):
            if g < ngrp and g not in pas:
                c0, gsz = starts[g], G_LIST[g]
                t = pap.tile([P, G, 2, CP], FP8, tag="pa", name=f"pa{g}")
                (eng or nc.sync).dma_start(
                    t[:, 0:gsz, :, :], pa_d[:, c0:c0 + gsz, :, :]
                )
                pas[g] = t

        # First transfers stay on Sync: the Scalar queue's early slots are
        # taken by the ACT table load, which then overlaps these transfers.
        issue_pa(0)
        issue_pa(1)

        coef_t = const.tile([P, NCHUNK], F32, tag="coef")
        nc.sync.dma_start(coef_t[:], coef_d)
        mask_t = const.tile([P, P], BF16, tag="mask")
        nc.sync.dma_start(mask_t[:], mask_d)

        gacc_t = accp.tile([P, NCHUNK], F32, tag="gacc")
        nc.vector.memset(gacc_t[:], 0.0)
        out_t = accp.tile([P, 8], F32, tag="out")
        nc.vector.memset(out_t[:], 0.0)

        # per-piece lse state: sume/lse/scr tiles sized to the piece
        grp_piece = {}
        psume, plse, pscr = [], [], []
        for pi, (g_lo, g_hi, nch, _, _) in enumerate(PIECES):
            psume.append(accp.tile([P, nch], F32, tag=f"sume{pi}", name=f"sume{pi}"))
            plse.append(accp.tile([P, nch], F32, tag=f"lse{pi}", name=f"lse{pi}"))
            pscr.append(accp.tile([P, nch], F32, tag=f"pscr{pi}", name=f"pscr{pi}"))
            for g in range(g_lo, g_hi):
                grp_piece[g] = pi

        def emit_lse_piece(pi):
            g_lo, g_hi, nch, _, col = PIECES[pi]
            p_lo = starts[g_lo]
            nc.scalar.activation(
                plse[pi][:], psume[pi][:], mybir.ActivationFunctionType.Ln
            )
            nc.vector.tensor_mul(
                pscr[pi][:], plse[pi][:], coef_t[:, p_lo:p_lo + nch]
            )
            nc.vector.tensor_reduce(
                out_t[:, col:col + 1],
                pscr[pi][:],
                axis=mybir.AxisListType.X,
                op=mybir.AluOpType.add,
            )

        spsum = [
            psum.tile([P, sz], F32, tag=f"sacc{m}", name=f"sacc{m}")
            for m, (off, sz) in enumerate(CTILES)
        ]

        first_mm = [True] * len(CTILES)
        for g in range(ngrp):
            c0, gsz = starts[g], G_LIST[g]
            issue_pa(g + 2)
            pa = pas.pop(g)
            pi = grp_piece[g]
            s_lo = c0 - starts[PIECES[pi][0]]

            # lse path: exp (ACT) then f16 halving tree (DVE, 2x mode)
            et = expp.tile([P, G, CP], F16, tag="exp")
            nc.scalar.activation(
                et[:, 0:gsz, :], pa[:, 0:gsz, 0, :],
                mybir.ActivationFunctionType.Exp,
            )
            tr = trp.tile([P, G, 300], F16, tag="tree")
            nc.vector.tensor_add(
                tr[:, 0:gsz, 0:160], et[:, 0:gsz, 0:160], et[:, 0:gsz, 160:320]
            )
            nc.vector.tensor_add(
                tr[:, 0:gsz, 160:240], tr[:, 0:gsz, 0:80], tr[:, 0:gsz, 80:160]
            )
            nc.vector.tensor_add(
                tr[:, 0:gsz, 240:280], tr[:, 0:gsz, 160:200], tr[:, 0:gsz, 200:240]
            )
            nc.vector.tensor_add(
                tr[:, 0:gsz, 280:300], tr[:, 0:gsz, 240:260], tr[:, 0:gsz, 260:280]
            )
            nc.vector.tensor_reduce(
                psume[pi][:, s_lo:s_lo + gsz],
                tr[:, 0:gsz, 280:300],
                axis=mybir.AxisListType.X,
                op=mybir.AluOpType.add,
            )
            for pj, (_, _, _, emit_after, _) in enumerate(PIECES):
                if emit_after == g:
                    emit_lse_piece(pj)

            # G path
            if g < pe_groups:
                for pr in range(gsz // 2):
                    jj = 2 * pr
                    last = g == pe_groups - 1 and pr == gsz // 2 - 1
                    for m, (off, sz) in enumerate(CTILES):
                        nc.tensor.matmul(
                            spsum[m][0:sz, :],
                            pa[:, jj:jj + 2, 1, off:off + sz],
                            pa[:, jj:jj + 2, 0, off:off + sz],
                            start=first_mm[m],
                            stop=last,
                            perf_mode=mybir.MatmulPerfMode.DoubleRow,
                        )
                        first_mm[m] = False
            else:
                scr = trp.tile([P, G, CP], F16, tag="dot")
                nc.vector.tensor_mul(
                    scr[:, 0:gsz, :], pa[:, 0:gsz, 0, :], pa[:, 0:gsz, 1, :]
                )
                nc.vector.tensor_reduce(
                    gacc_t[:, c0:c0 + gsz],
                    scr[:, 0:gsz, :],
                    axis=mybir.AxisListType.X,
                    op=mybir.AluOpType.add,
                )

        # final tail: last lse piece; G partials from PSUM diag blocks
        emit_lse_piece(len(PIECES) - 1)
        if pe_groups > 0:
            for m, (off, sz) in enumerate(CTILES):
                scrm = accp.tile([P, sz], F32, tag=f"scrm{m}")
                nc.vector.tensor_mul(
                    scrm[0:sz, :], spsum[m][0:sz, :], mask_t[0:sz, 0:sz]
                )
                nc.vector.tensor_reduce(
                    out_t[0:sz, 1 + m:2 + m],
                    scrm[0:sz, :],
                    axis=mybir.AxisListType.X,
                    op=mybir.AluOpType.add,
                )
        if pe_groups < ngrp:
            nc.vector.tensor_reduce(
                out_t[:, 4:5],
                gacc_t[:],
                axis=mybir.AxisListType.X,
                op=mybir.AluOpType.add,
            )
        nc.sync.dma_start(out_d, out_t[:])

    nc.compile()
    return nc


def host_inputs(pred, binned_color, knn_idx, knn_weights, weights):
    """Per-core input dicts. pred (B,C,H,W) f32; binned (B,1,H,W) int;
    knn_idx (C,K) int; knn_weights (C,K) f32; weights (C,) f32."""
    import ml_dtypes

    fp8 = ml_dtypes.float8_e4m3

    pred = np.asarray(pred, dtype=np.float32)
    binned = np.asarray(binned_color)
    knn_idx = np.asarray(knn_idx).astype(np.int64)
    knn_w = np.asarray(knn_weights, dtype=np.float32)
    wts = np.asarray(weights, dtype=np.float32)

    # A[t, c] = w[t] * sum_k knn_w[t,k] * [knn_idx[t,k] == c], padded to CP cols
    a_tab = np.zeros((C, CP), dtype=np.float32)
    rows = np.repeat(np.arange(C), K)
    cols = knn_idx.reshape(-1)
    vals = (wts[:, None] * knn_w).reshape(-1)
    np.add.at(a_tab, (rows, cols), vals)
    a_tab8 = a_tab.astype(fp8)

    coef_full = wts * knn_w.sum(axis=1)          # (C,)
    mask = np.eye(P, dtype=ml_dtypes.bfloat16)   # diag mask for G extraction

    in_maps = []
    for core in range(NCORES):
        bs = slice(core * BPC, (core + 1) * BPC)
        pm = np.full((PIX, CP), PAD_VAL, dtype=np.float32)
        pm[:, :C] = pred[bs].transpose(0, 2, 3, 1).reshape(PIX, C)
        tmap_pix = binned[bs, 0].reshape(PIX)
        # interleaved [P, NCHUNK, 2, CP]: slot 0 = pred, slot 1 = A-rows
        pa = np.empty((P, NCHUNK, 2, CP), dtype=fp8)
        pa[:, :, 0, :] = pm.reshape(NCHUNK, P, CP).transpose(1, 0, 2).astype(fp8)
        pa[:, :, 1, :] = a_tab8[tmap_pix].reshape(NCHUNK, P, CP).transpose(1, 0, 2)
        coef = np.ascontiguousarray(
            coef_full[tmap_pix].reshape(NCHUNK, P).T
        ).astype(np.float32)
        in_maps.append(
            {
                "pa_t": pa,
                "coef_t": coef,
                "mask_t": mask,
            }
        )
    return in_maps


def combine_outputs(core_outs):
    """core_outs: list of [128, 8] f32 arrays -> scalar loss."""
    total = 0.0
    for o in core_outs:
        o = o.astype(np.float64)
        lsec = o[:, 0].sum() + o[:, 5].sum() + o[:, 6].sum()
        g = o[:, 1:5].sum()
        total += lsec - g
    return np.array(total / NTOT, dtype=np.float32)


_NC_CACHE = None


def kernel(pred, _color, binned_color, knn_idx, knn_weights, weights):
    global _NC_CACHE
    if _NC_CACHE is None:
        _NC_CACHE = build_program()
    nc = _NC_CACHE
    in_maps = host_inputs(pred, binned_color, knn_idx, knn_weights, weights)
    res = bass_utils.run_bass_kernel_spmd(nc, in_maps, core_ids=list(range(NCORES)))
    outs = [res.results[i]["out"] for i in range(NCORES)]
    return combine_outputs(outs)


if __name__ == "__main__":
    import jax
    import reference

    with jax.default_device(jax.devices("cpu")[0]):
        inputs = reference.setup_inputs()
        inputs = {k: np.asarray(jax.device_get(v)) for k, v in inputs.items()}
    got = kernel(**inputs)
    print("kernel loss:", got)

